# revision 29
# baseline (speedup 1.0000x reference)
"""Bass/Tile Trainium2 kernel for a single attention head.

Problem: B=4, S=4096, D_IN=1024, D=128.
  q = query @ Wq + bq ; k = key @ Wk + bk ; v = value @ Wv + bv
  out = softmax(q k^T / sqrt(D)) v

Sharding: 8 cores; core c handles batch b=c//2, half h=c%2: it owns
query/key/value rows [h*2048, (h+1)*2048) of batch b. Each core projects
its own K/V half, the core pair exchanges projected K^T / V via pairwise
AllGathers, and each core then runs attention for its 2048 queries over
all 4096 keys.

Math notes:
 - softmax over keys is invariant to per-query-row constants, so the
   bk bias term is dropped.
 - logits are small (|logit| < ~3 for randn inputs), so exp() without
   max-subtraction is numerically safe.
 - A ones-column appended to V gives the softmax denominator in the
   same PSUM accumulation as P@V.

Schedule (v2): the PE instruction stream is in-order, so emission order
IS the schedule. Baseline lost ~35us to (a) the V exchange running with
nothing else in flight, (b) scores stalling on late K-exchange parts,
and (c) an 8.5us cold start. v2:
 - fp32->bf16 cast DMAs only run on the gpsimd queue, which is the feed
   bottleneck: K and V blocks stream there (with the wk/wq/wv loads);
   the four Q blocks load as raw fp32 on the otherwise-idle vector
   queue and are transposed on the PE in fp32 (2 cycles/row, bf16 out).
 - The K exchange is split into 4 column parts, each launched the
   moment its source block is projected, with hops spread over the
   sync/vector queues; scores start when part 0 lands (~20us) and
   consume parts in arrival order (SC_ORDER).
 - V projection and the remaining Q blocks weave one nugget per score
   step into the ACT-paced score phases (scores only need ~40% of PE
   there). The V exchange goes out in 2 parts; AV iterates keys in
   part-arrival order (AV_ORDER) so P@V chunks weave into the last
   score phases as soon as part 0 lands.
 - V^T tiles are DMA'd straight from their transpose PSUM bank into the
   DRAM exchange staging (no SBUF copy).
 - PE is pre-warmed with identity transposes (p-state ramp) and the ACT
   exp table is pre-loaded during the initial DMA window.
Score tiles are computed transposed (S^T[k, q], keys on partitions) so
exp output chunks (P^T) serve directly as AV stationary operands.

PSUM budget (8 banks):
  scpool: 2 bufs x [128,1024]f32 = 4 banks (score tiles, exp sources)
  pspool: 4 bufs x 1 bank, shared tag rotating: proj transposes (bf16),
          proj outputs, and the AV accumulators packed 3+1 per bank
          (matmul start=True zeroes the whole bank).
"""

import math
import sys

import numpy as np

for _p in ("/opt/trn_rl_repo", "/root/.axon_site/_ro/trn_rl_repo"):
    if _p not in sys.path:
        sys.path.append(_p)

import concourse.bass as bass  # noqa: E402
import concourse.mybir as mybir  # noqa: E402
import concourse.tile as tile  # noqa: E402
from concourse import bacc  # noqa: E402
from concourse.bass_utils import run_bass_kernel_spmd  # noqa: E402
from concourse.masks import make_identity  # noqa: E402

FP32 = mybir.dt.float32
BF16 = mybir.dt.bfloat16

B, S, D_IN, D = 4, 4096, 1024, 128
N_CORES = 8
VSLOT = 132  # per-key-tile slot width for v_aug (128 v cols + 1 ones + pad)
WARMUP_TRANSPOSES = 38


def build_program(nc, sq, skv_local, n_cores=8, pair_split=True, reps=1):
    """Emit the Tile program.

    sq: query rows per core. skv_local: kv rows this core projects.
    pair_split: exchange projected K^T / V across core pairs via
    AllGather (total keys = 2*skv_local); otherwise each core handles
    skv_local keys standalone.
    reps > 1 wraps the whole computation in an on-device For_i loop for
    benchmarking (only valid with pair_split=False or "mock":
    collectives cannot sit inside control flow).
    """
    assert reps == 1 or pair_split in (False, "mock")
    skv_tot = 2 * skv_local if pair_split else skv_local

    q_in = nc.dram_tensor("q_in", [sq, D_IN], FP32, kind="ExternalInput")
    k_in = nc.dram_tensor("k_in", [skv_local, D_IN], FP32, kind="ExternalInput")
    v_in = nc.dram_tensor("v_in", [skv_local, D_IN], FP32, kind="ExternalInput")
    wq = nc.dram_tensor("wq", [D_IN, D], FP32, kind="ExternalInput")
    wk = nc.dram_tensor("wk", [D_IN, D], FP32, kind="ExternalInput")
    wv = nc.dram_tensor("wv", [D_IN, D], FP32, kind="ExternalInput")
    bq = nc.dram_tensor("bq", [D, 1], FP32, kind="ExternalInput")
    bv = nc.dram_tensor("bv", [D, 1], FP32, kind="ExternalInput")
    out = nc.dram_tensor("out", [sq, D], FP32, kind="ExternalOutput")

    n_ic = D_IN // 128  # contraction chunks
    nkt_loc = skv_local // 128
    nkt = skv_tot // 128
    nqb = sq // 512
    scale = 1.0 / math.sqrt(D)

    with tile.TileContext(nc) as tc:
        with (
            tc.tile_pool(name="const", bufs=1) as cpool,
            tc.tile_pool(name="wts", bufs=1) as wpool,
            tc.tile_pool(name="projout", bufs=1) as opool,
            tc.tile_pool(name="xload", bufs=5) as xpool,
            tc.tile_pool(name="x32", bufs=2) as x32pool,
            tc.tile_pool(name="xt", bufs=6) as xtpool,
            tc.tile_pool(name="sc", bufs=2, space="PSUM") as scpool,
            tc.tile_pool(name="ps", bufs=4, space="PSUM") as pspool,
            tc.tile_pool(name="pt", bufs=3) as ptpool,
            tc.tile_pool(name="fin", bufs=2) as finpool,
            tc.tile_pool(name="dram", bufs=1, space="DRAM") as dpool,
        ):
            def emit_body():
                ident = cpool.tile([128, 128], BF16)
                make_identity(nc, ident[:])
                ident32 = cpool.tile([128, 128], FP32, tag="ident32")
                make_identity(nc, ident32[:])

                # Each W [1024, 128] loads as one cast-DMA into a [128, 8*128]
                # tile; chunk ic lives at cols ic*128:(ic+1)*128 with the
                # contraction index on partitions. Casts are gpsimd-only.
                w_sb = {}

                def load_w(name, wdram):
                    t = wpool.tile([128, n_ic * D], BF16, tag=f"w_{name}")
                    nc.gpsimd.dma_start(
                        out=t[:].rearrange("p (c d) -> p c d", c=n_ic),
                        in_=wdram[:, :].rearrange("(c p) d -> p c d", c=n_ic),
                    )
                    w_sb[name] = [t[:, ic * D : (ic + 1) * D] for ic in range(n_ic)]

                qT = opool.tile([128, sq], BF16, tag="qT")
                kTl = opool.tile([128, skv_local], BF16, tag="kTl")
                vTl = opool.tile([128, skv_local], BF16, tag="vTl")
                vtl = opool.tile([128, nkt_loc * 128], BF16, tag="vtl")
                kT = opool.tile([128, skv_tot], BF16, tag="kT")
                vfull = opool.tile([128, nkt * VSLOT], BF16, tag="vfull")

                def kslice(kt):
                    if not pair_split:
                        return kTl[:, kt * 128 : (kt + 1) * 128]
                    return kT[:, kt * 128 : (kt + 1) * 128]

                class Proj:
                    """Projection with DMA issue decoupled from compute.

                    Blocks in fp32_blocks load uncast (issuable on the
                    sync/scalar hwdge queues, avoiding the gpsimd-only cast
                    queue) and are transposed on the PE in fp32 (2
                    cycles/row); the transpose still writes bf16 to PSUM so
                    everything downstream is identical.
                    """

                    def __init__(self, x_dram, s_len, w_name, dstT, bias_ap,
                                 fp32_blocks=frozenset()):
                        self.x_dram = x_dram
                        self.nblk = s_len // 512
                        self.w_name = w_name
                        self.dstT = dstT
                        self.bias_ap = bias_ap
                        self.fp32_blocks = fp32_blocks
                        self.xs = [None] * self.nblk

                    def issue(self, sb, eng):
                        if sb in self.fp32_blocks:
                            xs = x32pool.tile([128, 4 * D_IN], FP32, tag="x32")
                        else:
                            xs = xpool.tile([128, 4 * D_IN], BF16, tag="xload")
                        r0 = sb * 512
                        eng.dma_start(
                            out=xs[:].rearrange("p (s i) -> p s i", s=4),
                            in_=self.x_dram[r0 : r0 + 512, :].rearrange(
                                "(s p) i -> p s i", s=4
                            ),
                        )
                        self.xs[sb] = xs

                    def gen(self):
                        for sb in range(self.nblk):
                            fp32 = sb in self.fp32_blocks
                            w_tiles = w_sb[self.w_name]
                            xs = self.xs[sb]
                            assert xs is not None, "xs DMA not issued"
                            xts = []
                            for icp in range(n_ic // 2):
                                xt_sb = xtpool.tile([128, 1024], BF16, tag="xt")
                                if fp32:
                                    # fp32 transposes must write fp32 PSUM:
                                    # half-width nuggets, one bank each.
                                    for half in range(2):
                                        ic = 2 * icp + half
                                        tp = pspool.tile(
                                            [128, 512], FP32, tag="ps"
                                        )
                                        for ss in range(4):
                                            nc.tensor.transpose(
                                                tp[:, ss * 128 : (ss + 1) * 128],
                                                xs[
                                                    :,
                                                    ss * D_IN
                                                    + ic * 128 : ss * D_IN
                                                    + (ic + 1) * 128,
                                                ],
                                                ident32[:],
                                            )
                                        nc.vector.tensor_copy(
                                            xt_sb[
                                                :, half * 512 : (half + 1) * 512
                                            ],
                                            tp[:],
                                        )
                                        yield
                                else:
                                    # two bf16 chunks share one PSUM bank
                                    tp = pspool.tile([128, 1024], BF16, tag="ps")
                                    for half in range(2):
                                        ic = 2 * icp + half
                                        for ss in range(4):
                                            nc.tensor.transpose(
                                                tp[
                                                    :,
                                                    half * 512
                                                    + ss * 128 : half * 512
                                                    + (ss + 1) * 128,
                                                ],
                                                xs[
                                                    :,
                                                    ss * D_IN
                                                    + ic * 128 : ss * D_IN
                                                    + (ic + 1) * 128,
                                                ],
                                                ident[:],
                                            )
                                    nc.vector.tensor_copy(xt_sb[:], tp[:])
                                    yield
                                xts.append(xt_sb[:, 0:512])
                                xts.append(xt_sb[:, 512:1024])
                            pp = pspool.tile([128, 512], FP32, tag="ps")
                            for ic in range(n_ic):
                                nc.tensor.matmul(
                                    pp[:],
                                    w_tiles[ic],
                                    xts[ic],
                                    start=(ic == 0),
                                    stop=(ic == n_ic - 1),
                                )
                            dst = self.dstT[:, sb * 512 : (sb + 1) * 512]
                            if self.bias_ap is None:
                                nc.vector.tensor_copy(dst, pp[:])
                            else:
                                nc.vector.tensor_scalar_add(
                                    dst, pp[:], self.bias_ap
                                )
                            yield

                # DRAM staging for the V exchange (one [128,512] DMA per
                # vtrans nugget, via the vtl SBUF staging tile). One
                # contiguous DRAM tile per exchange part: collectives
                # require contiguous input patterns.
                cc_in_v = [
                    dpool.tile(
                        [128, 4 * 128], BF16, tag=f"cc_in_v{p}",
                        name=f"cc_in_v{p}",
                    )
                    for p in range(max(nkt_loc // 4, 1))
                ]

                def vtrans_gen():
                    """Natural-layout local v tiles from vT (PE transposes),
                    shipped to DRAM exchange staging (pair modes) or copied
                    into vfull's local slots (standalone)."""
                    for kt4 in range(nkt_loc // 4):
                        tp = pspool.tile([128, 1024], BF16, tag="ps")
                        for j in range(4):
                            kt = kt4 * 4 + j
                            nc.tensor.transpose(
                                tp[:, j * 128 : (j + 1) * 128],
                                vTl[:, kt * 128 : (kt + 1) * 128],
                                ident[:],
                            )
                        if pair_split:
                            nc.vector.tensor_copy(
                                vtl[:, kt4 * 512 : (kt4 + 1) * 512], tp[:, 0:512]
                            )
                            nc.sync.dma_start(
                                out=cc_in_v[kt4][:, :],
                                in_=vtl[:, kt4 * 512 : (kt4 + 1) * 512],
                            )
                        else:
                            nc.vector.tensor_copy(
                                vfull[
                                    :, kt4 * 4 * VSLOT : (kt4 * 4 + 4) * VSLOT
                                ].rearrange("p (j s) -> p j s", j=4)[:, :, 0:128],
                                tp[:, 0:512].rearrange(
                                    "p (j d) -> p j d", j=4
                                ),
                            )
                        yield

                # P^T staging: one tile per query block, 16 ktp x 1024 cols.
                pt_tiles = {}

                def pt_tile(qb):
                    if qb not in pt_tiles:
                        t = ptpool.tile(
                            [128, (nkt // 2) * 1024], BF16, tag="pt",
                            name=f"pt{qb}",
                        )
                        pt_tiles[qb] = t
                    return pt_tiles[qb]

                # K-exchange part p carries kTl cols [p*512,(p+1)*512) and
                # lands as score pairs {2p,2p+1} (rank 0) and {8+2p,8+2p+1}
                # (rank 1). Order score steps by part arrival.
                if pair_split:
                    SC_ORDER = [
                        ktp
                        for p in range(2)
                        for ktp in (
                            4 * p, 4 * p + 1, 4 * p + 2, 4 * p + 3,
                            8 + 4 * p, 8 + 4 * p + 1, 8 + 4 * p + 2,
                            8 + 4 * p + 3,
                        )
                    ]
                else:
                    SC_ORDER = list(range(nkt // 2))

                # V-exchange part p carries local key tiles [8p, 8p+8) and
                # lands as global tiles {8p..} (rank 0) and {16+8p..}
                # (rank 1): iterate AV over keys in part-arrival order.
                if pair_split:
                    AV_ORDER = [
                        kt
                        for p in range(4)
                        for kt in list(range(4 * p, 4 * p + 4))
                        + list(range(16 + 4 * p, 16 + 4 * p + 4))
                    ]
                else:
                    AV_ORDER = list(range(nkt))

                def sc_gen(qb):
                    """Scores + exp for one query block; yields per ktp."""
                    pt = pt_tile(qb)
                    for ktp in SC_ORDER:
                        sc = scpool.tile([128, 1024], FP32, tag="sc")
                        for half in range(2):
                            kt = 2 * ktp + half
                            nc.tensor.matmul(
                                sc[:, half * 512 : (half + 1) * 512],
                                kslice(kt),
                                qT[:, qb * 512 : (qb + 1) * 512],
                                start=True,
                                stop=True,
                            )
                        nc.scalar.activation(
                            pt[:, ktp * 1024 : (ktp + 1) * 1024],
                            sc[:],
                            mybir.ActivationFunctionType.Exp,
                            bias=0.0,
                            scale=scale,
                        )
                        yield

                def av_gen(qb):
                    """P@V accumulation + normalize + store for one query
                    block; yields every 2 key tiles."""
                    pt = pt_tile(qb)
                    # accumulators packed 3+1 into two banks, exploiting that
                    # matmul start=True zeroes the WHOLE bank: qs0's start
                    # zeroes bank A (incl. qs1/qs2 regions, which then
                    # accumulate with start=False onto zeros); qs3's start
                    # zeroes bank B.
                    avA = pspool.tile([128, 3 * 129], FP32, tag="ps", name="avA")
                    avB = pspool.tile([128, 129], FP32, tag="ps", name="avB")
                    avs = [
                        avA[:, 0:129],
                        avA[:, 129:258],
                        avA[:, 258:387],
                        avB[:, 0:129],
                    ]
                    for i, kt in enumerate(AV_ORDER):
                        ktp, half = kt // 2, kt % 2
                        for qs in range(4):
                            nc.tensor.matmul(
                                avs[qs],
                                pt[
                                    :,
                                    ktp * 1024
                                    + half * 512
                                    + qs * 128 : ktp * 1024
                                    + half * 512
                                    + (qs + 1) * 128,
                                ],
                                vfull[:, kt * VSLOT : kt * VSLOT + 129],
                                start=(i == 0 and qs in (0, 3)),
                                stop=(i == nkt - 1),
                                skip_group_check=(i == 0 and qs in (1, 2)),
                            )
                        if i % 2 == 1:
                            yield
                    obuf = finpool.tile([128, 4 * D], FP32, tag="obuf")
                    for qs in range(4):
                        rec = finpool.tile([128, 1], FP32, tag="rec")
                        nc.vector.reciprocal(rec[:], avs[qs][:, 128:129])
                        nc.vector.tensor_scalar_mul(
                            obuf[:, qs * D : (qs + 1) * D], avs[qs][:, 0:128], rec[:]
                        )
                    r0 = qb * 512
                    nc.sync.dma_start(
                        out=out[r0 : r0 + 512, :].rearrange("(s p) d -> p s d", s=4),
                        in_=obuf[:].rearrange("p (s d) -> p s d", s=4),
                    )
                    yield

                groups = [[2 * i, 2 * i + 1] for i in range(n_cores // 2)]

                def exchange(src_ap, src_is_dram, unpack_aps, tag, eng_a, eng_b):
                    """Ship a [128, w] slice across the core pair;
                    unpack_aps[h] receives rank h's copy. Mock mode bounces
                    the data through DRAM (equivalent transfer volume) with
                    hops split across two queues so parts pipeline."""
                    w = src_ap.shape[-1]
                    if src_is_dram:
                        cc_in = src_ap
                    else:
                        t = dpool.tile(
                            [128, w], BF16, tag=f"cc_in_{tag}",
                            name=f"cc_in_{tag}",
                        )
                        eng_a.dma_start(out=t[:], in_=src_ap)
                        cc_in = t[:]
                    if pair_split == "mock":
                        cc_b = dpool.tile(
                            [128, w], BF16, tag=f"cc_b_{tag}", name=f"cc_b_{tag}"
                        )
                        hw = w // 2
                        eng_a.dma_start(out=cc_b[:, 0:hw], in_=cc_in[:, 0:hw])
                        eng_b.dma_start(out=cc_b[:, hw:w], in_=cc_in[:, hw:w])
                        halves = [cc_b[:], cc_b[:]]
                    else:
                        cc_out = dpool.tile(
                            [2, 128, w], BF16, tag=f"cc_o_{tag}",
                            name=f"cc_o_{tag}",
                        )
                        nc.gpsimd.collective_compute(
                            "AllGather",
                            mybir.AluOpType.bypass,
                            replica_groups=groups,
                            ins=[cc_in.opt()],
                            outs=[cc_out.opt()],
                        )
                        halves = [cc_out[0], cc_out[1]]
                    for h, eng in ((0, eng_a), (1, eng_b)):
                        eng.dma_start(out=unpack_aps[h], in_=halves[h])

                def k_exchange(part):
                    """Exchange part p = 1024 projected-K^T columns. The
                    scalar (ACT) queue only carries hops that finish before
                    the exp stream starts."""
                    c0 = part * 1024
                    eng_a, eng_b = [
                        (nc.sync, nc.scalar),
                        (nc.scalar, nc.sync),
                    ][part]
                    exchange(
                        kTl[:, c0 : c0 + 1024],
                        False,
                        [
                            kT[:, h * skv_local + c0 : h * skv_local + c0 + 1024]
                            for h in range(2)
                        ],
                        f"k{part}",
                        eng_a,
                        eng_b,
                    )

                def v_exchange(part):
                    """Exchange part p = 4 local natural-layout v tiles
                    (one vtrans nugget). Hops on sync: scalar carries exp,
                    gpsimd carries the input casts."""
                    exchange(
                        cc_in_v[part][:, :],
                        True,
                        [
                            vfull[
                                :,
                                (h * nkt_loc + part * 4)
                                * VSLOT : (h * nkt_loc + part * 4 + 4)
                                * VSLOT,
                            ].rearrange("p (j s) -> p j s", j=4)[:, :, 0:128]
                            for h in range(2)
                        ],
                        f"v{part}",
                        nc.sync,
                        nc.sync,
                    )

                def drain(g):
                    for _ in g:
                        pass

                def advance(g, n):
                    took = 0
                    try:
                        for _ in range(n):
                            next(g)
                            took += 1
                    except StopIteration:
                        pass
                    return took

                # ---- emission schedule ----
                bq_sb = cpool.tile([128, 1], FP32, tag="bq")
                bv_sb = cpool.tile([128, 1], FP32, tag="bv")
                kp = Proj(k_in, skv_local, "wk", kTl, None)
                qp = Proj(q_in, sq, "wq", qT, bq_sb[:])
                vp = Proj(v_in, skv_local, "wv", vTl, bv_sb[:])
                kgen, qgen, vgen = kp.gen(), qp.gen(), vp.gen()
                vtrans = vtrans_gen()
                scs = [sc_gen(qb) for qb in range(nqb)]
                avs_g = [av_gen(qb) for qb in range(nqb)]

                # t0 DMA queueing, all casts on the single gpsimd queue.
                # The device is aggregate-HBM-bound (~275 GB/s effective
                # across 8 cores), so one queue is as good as three; order
                # is what matters. Blocks land every ~7us: K first (its
                # exchange gates all scores), Q0 next, then Q1/Q2 (each
                # gates its score phase), V spread so each V-exchange part
                # launches as its data lands, Q3 before V3.
                load_w("wk", wk)
                kp.issue(0, nc.gpsimd)
                kp.issue(1, nc.gpsimd)
                load_w("wq", wq)
                qp.issue(0, nc.gpsimd)
                kp.issue(2, nc.gpsimd)
                kp.issue(3, nc.gpsimd)
                qp.issue(1, nc.gpsimd)
                qp.issue(2, nc.gpsimd)
                vp.issue(0, nc.gpsimd)
                load_w("wv", wv)
                nc.sync.dma_start(out=bq_sb[:], in_=bq[:, :])
                nc.sync.dma_start(out=bv_sb[:], in_=bv[:, :])

                # warmups during the initial DMA window: PE p-state ramp via
                # identity transposes; ACT exp-table preload (1283ns) off the
                # critical exp stream.
                wps = pspool.tile([128, 512], BF16, tag="ps", name="warm")
                for i in range(WARMUP_TRANSPOSES):
                    nc.tensor.transpose(
                        wps[:, (i % 4) * 128 : (i % 4 + 1) * 128],
                        ident[:],
                        ident[:],
                    )
                actwarm = cpool.tile([128, 1], FP32, tag="actwarm")
                nc.vector.memset(actwarm[:], 0.0)
                actwarm2 = cpool.tile([128, 1], FP32, tag="actwarm2")
                nc.scalar.activation(
                    actwarm2[:],
                    actwarm[:],
                    mybir.ActivationFunctionType.Exp,
                    bias=0.0,
                    scale=1.0,
                )
                # ones columns of v_aug, one strided memset; disjoint from
                # the unpack DMAs' [0:128) slot regions.
                nc.vector.memset(
                    vfull[:, :].rearrange("p (j s) -> p j s", j=nkt)[
                        :, :, 128:129
                    ],
                    1.0,
                )

                # K blocks; exchange part A (cols 0:1024) after K1, part B
                # after K3. Q0 projects between them so qb0 scores start as
                # soon as part A lands.
                advance(kgen, 5)  # K block 0
                advance(kgen, 5)  # K block 1
                if pair_split:
                    k_exchange(0)
                advance(qgen, 5)  # Q block 0
                advance(kgen, 5)  # K block 2
                drain(kgen)  # K block 3
                if pair_split:
                    k_exchange(1)

                # qb0 scores: input-bound window, no fills (later blocks
                # haven't landed yet); the part-B tail of qb0 overlaps the
                # Q1 arrival gap.
                drain(scs[0])
                # Q1 projection gates qb1's scores.
                advance(qgen, 5)  # Q1
                vp.issue(1, nc.gpsimd)
                # qb1 scores: weave V0/V1 as they land; first two V-exchange
                # parts go out per vtrans nugget; Q2 last (lands ~54us).
                for i in range(5):
                    advance(scs[1], 1)
                    advance(vgen, 1)  # V0
                vp.issue(2, nc.gpsimd)
                for i in range(5):
                    advance(scs[1], 1)
                    advance(vgen, 1)  # V1
                qp.issue(3, nc.gpsimd)
                for i in range(2):
                    advance(scs[1], 1)
                    advance(vtrans, 1)  # vtrans n0, n1
                    if pair_split:
                        v_exchange(i)
                for i in range(4):
                    advance(scs[1], 1)
                    advance(qgen, 1)  # Q2 n0-3
                advance(qgen, 1)  # Q2 n4 (gates qb2 scores)
                vp.issue(3, nc.gpsimd)
                # qb2 scores: V2/V3 land here; exchange parts 2-3 per
                # nugget; AV(qb0) part-0 chunks weave in once vfull part 0
                # lands (~80us).
                for i in range(5):
                    advance(scs[2], 1)
                    advance(vgen, 1)  # V2
                for i in range(5):
                    advance(scs[2], 1)
                    advance(vgen, 1)  # V3
                for i in range(2):
                    advance(scs[2], 1)
                    advance(vtrans, 1)  # vtrans n2, n3
                    if pair_split:
                        v_exchange(2 + i)
                drain(vtrans)
                drain(vgen)
                for i in range(4):
                    advance(scs[2], 1)
                    advance(avs_g[0], 1)
                # Q3 projection gates qb3's scores.
                drain(qgen)  # Q3
                # qb3 scores: drain AV(qb0) at double rate (frees the pt
                # buffer qb3's exps rotate onto), then AV(qb1).
                for i in range(16):
                    advance(scs[3], 1)
                    if i < 7:
                        advance(avs_g[0], 2)
                    else:
                        advance(avs_g[1], 1)
                for g in avs_g:
                    drain(g)

            if reps > 1:
                hint = (
                    mybir.EngineType.PE,
                    mybir.EngineType.DVE,
                    mybir.EngineType.Activation,
                    mybir.EngineType.SP,
                    mybir.EngineType.Pool,
                )
                with tc.For_i(0, reps, 1, hint_engines=hint):
                    emit_body()
            else:
                emit_body()

    return nc


def build_graph(
    sq=S // 2, skv_local=S // 2, n_cores=N_CORES, pair_split=True, reps=1
):
    nc = bacc.Bacc(
        "TRN2",
        target_bir_lowering=False,
        debug=False,
        enable_asserts=True,
        num_devices=n_cores,
    )
    build_program(
        nc, sq, skv_local, n_cores=n_cores, pair_split=pair_split, reps=reps
    )
    nc.compile()
    return nc


_NC = None


def _get_nc():
    global _NC
    if _NC is None:
        _NC = build_graph()
    return _NC


def make_in_maps(query, key, value, Wq, bq, Wk, bk, Wv, bv):
    query = np.asarray(query, dtype=np.float32)
    key = np.asarray(key, dtype=np.float32)
    value = np.asarray(value, dtype=np.float32)
    Wq = np.ascontiguousarray(np.asarray(Wq, dtype=np.float32))
    Wk = np.ascontiguousarray(np.asarray(Wk, dtype=np.float32))
    Wv = np.ascontiguousarray(np.asarray(Wv, dtype=np.float32))
    bq2 = np.ascontiguousarray(np.asarray(bq, np.float32).reshape(D, 1))
    bv2 = np.ascontiguousarray(np.asarray(bv, np.float32).reshape(D, 1))
    sq = S // 2
    in_maps = []
    for c in range(N_CORES):
        b, h = c // 2, c % 2
        in_maps.append(
            {
                "q_in": np.ascontiguousarray(query[b, h * sq : (h + 1) * sq, :]),
                "k_in": np.ascontiguousarray(key[b, h * sq : (h + 1) * sq, :]),
                "v_in": np.ascontiguousarray(value[b, h * sq : (h + 1) * sq, :]),
                "wq": Wq,
                "wk": Wk,
                "wv": Wv,
                "bq": bq2,
                "bv": bv2,
            }
        )
    return in_maps


def assemble_out(results):
    sq = S // 2
    out = np.empty((B, S, D), np.float32)
    for c in range(N_CORES):
        b, h = c // 2, c % 2
        out[b, h * sq : (h + 1) * sq, :] = results[c]["out"]
    return out


def kernel(query, key, value, Wq, bq, Wk, bk, Wv, bv):
    nc = _get_nc()
    in_maps = make_in_maps(query, key, value, Wq, bq, Wk, bk, Wv, bv)
    res = run_bass_kernel_spmd(nc, in_maps, core_ids=list(range(N_CORES)))
    return assemble_out(res.results)


# revision 30
# speedup vs baseline: 1.0157x; 1.0157x over previous
"""Bass/Tile Trainium2 kernel for a single attention head.

Problem: B=4, S=4096, D_IN=1024, D=128.
  q = query @ Wq + bq ; k = key @ Wk + bk ; v = value @ Wv + bv
  out = softmax(q k^T / sqrt(D)) v

Sharding: 8 cores; core c handles batch b=c//2, half h=c%2: it owns
query/key/value rows [h*2048, (h+1)*2048) of batch b. Each core projects
its own K/V half, the core pair exchanges projected K^T / V via pairwise
AllGathers, and each core then runs attention for its 2048 queries over
all 4096 keys.

Math notes:
 - softmax over keys is invariant to per-query-row constants, so the
   bk bias term is dropped.
 - logits are small (|logit| < ~3 for randn inputs), so exp() without
   max-subtraction is numerically safe.
 - A ones-column appended to V gives the softmax denominator in the
   same PSUM accumulation as P@V.

Schedule (v2): the PE instruction stream is in-order, so emission order
IS the schedule. Baseline lost ~35us to (a) the V exchange running with
nothing else in flight, (b) scores stalling on late K-exchange parts,
and (c) an 8.5us cold start. v2:
 - fp32->bf16 cast DMAs only run on the gpsimd queue, which is the feed
   bottleneck: K and V blocks stream there (with the wk/wq/wv loads);
   the four Q blocks load as raw fp32 on the otherwise-idle vector
   queue and are transposed on the PE in fp32 (2 cycles/row, bf16 out).
 - The K exchange is split into 4 column parts, each launched the
   moment its source block is projected, with hops spread over the
   sync/vector queues; scores start when part 0 lands (~20us) and
   consume parts in arrival order (SC_ORDER).
 - V projection and the remaining Q blocks weave one nugget per score
   step into the ACT-paced score phases (scores only need ~40% of PE
   there). The V exchange goes out in 2 parts; AV iterates keys in
   part-arrival order (AV_ORDER) so P@V chunks weave into the last
   score phases as soon as part 0 lands.
 - V^T tiles are DMA'd straight from their transpose PSUM bank into the
   DRAM exchange staging (no SBUF copy).
 - PE is pre-warmed with identity transposes (p-state ramp) and the ACT
   exp table is pre-loaded during the initial DMA window.
Score tiles are computed transposed (S^T[k, q], keys on partitions) so
exp output chunks (P^T) serve directly as AV stationary operands.

PSUM budget (8 banks):
  scpool: 2 bufs x [128,1024]f32 = 4 banks (score tiles, exp sources)
  pspool: 4 bufs x 1 bank, shared tag rotating: proj transposes (bf16),
          proj outputs, and the AV accumulators packed 3+1 per bank
          (matmul start=True zeroes the whole bank).
"""

import math
import sys

import numpy as np

for _p in ("/opt/trn_rl_repo", "/root/.axon_site/_ro/trn_rl_repo"):
    if _p not in sys.path:
        sys.path.append(_p)

import concourse.bass as bass  # noqa: E402
import concourse.mybir as mybir  # noqa: E402
import concourse.tile as tile  # noqa: E402
from concourse import bacc  # noqa: E402
from concourse.bass_utils import run_bass_kernel_spmd  # noqa: E402
from concourse.masks import make_identity  # noqa: E402

FP32 = mybir.dt.float32
BF16 = mybir.dt.bfloat16

B, S, D_IN, D = 4, 4096, 1024, 128
N_CORES = 8
VSLOT = 132  # per-key-tile slot width for v_aug (128 v cols + 1 ones + pad)
WARMUP_TRANSPOSES = 0


def build_program(nc, sq, skv_local, n_cores=8, pair_split=True, reps=1):
    """Emit the Tile program.

    sq: query rows per core. skv_local: kv rows this core projects.
    pair_split: exchange projected K^T / V across core pairs via
    AllGather (total keys = 2*skv_local); otherwise each core handles
    skv_local keys standalone.
    reps > 1 wraps the whole computation in an on-device For_i loop for
    benchmarking (only valid with pair_split=False or "mock":
    collectives cannot sit inside control flow).
    """
    assert reps == 1 or pair_split in (False, "mock")
    skv_tot = 2 * skv_local if pair_split else skv_local

    q_in = nc.dram_tensor("q_in", [sq, D_IN], FP32, kind="ExternalInput")
    k_in = nc.dram_tensor("k_in", [skv_local, D_IN], FP32, kind="ExternalInput")
    v_in = nc.dram_tensor("v_in", [skv_local, D_IN], FP32, kind="ExternalInput")
    wq = nc.dram_tensor("wq", [D_IN, D], FP32, kind="ExternalInput")
    wk = nc.dram_tensor("wk", [D_IN, D], FP32, kind="ExternalInput")
    wv = nc.dram_tensor("wv", [D_IN, D], FP32, kind="ExternalInput")
    bq = nc.dram_tensor("bq", [D, 1], FP32, kind="ExternalInput")
    bv = nc.dram_tensor("bv", [D, 1], FP32, kind="ExternalInput")
    out = nc.dram_tensor("out", [sq, D], FP32, kind="ExternalOutput")

    n_ic = D_IN // 128  # contraction chunks
    nkt_loc = skv_local // 128
    nkt = skv_tot // 128
    nqb = sq // 512
    scale = 1.0 / math.sqrt(D)

    with tile.TileContext(nc) as tc:
        with (
            tc.tile_pool(name="const", bufs=1) as cpool,
            tc.tile_pool(name="wts", bufs=1) as wpool,
            tc.tile_pool(name="projout", bufs=1) as opool,
            tc.tile_pool(name="xload", bufs=5) as xpool,
            tc.tile_pool(name="x32", bufs=2) as x32pool,
            tc.tile_pool(name="xt", bufs=6) as xtpool,
            tc.tile_pool(name="sc", bufs=2, space="PSUM") as scpool,
            tc.tile_pool(name="ps", bufs=4, space="PSUM") as pspool,
            tc.tile_pool(name="pt", bufs=3) as ptpool,
            tc.tile_pool(name="fin", bufs=2) as finpool,
            tc.tile_pool(name="dram", bufs=1, space="DRAM") as dpool,
        ):
            def emit_body():
                ident = cpool.tile([128, 128], BF16)
                make_identity(nc, ident[:])
                ident32 = cpool.tile([128, 128], FP32, tag="ident32")
                make_identity(nc, ident32[:])

                # Each W [1024, 128] loads as one cast-DMA into a [128, 8*128]
                # tile; chunk ic lives at cols ic*128:(ic+1)*128 with the
                # contraction index on partitions. Casts are gpsimd-only.
                w_sb = {}

                def load_w(name, wdram):
                    t = wpool.tile([128, n_ic * D], BF16, tag=f"w_{name}")
                    nc.gpsimd.dma_start(
                        out=t[:].rearrange("p (c d) -> p c d", c=n_ic),
                        in_=wdram[:, :].rearrange("(c p) d -> p c d", c=n_ic),
                    )
                    w_sb[name] = [t[:, ic * D : (ic + 1) * D] for ic in range(n_ic)]

                qT = opool.tile([128, sq], BF16, tag="qT")
                kTl = opool.tile([128, skv_local], BF16, tag="kTl")
                vTl = opool.tile([128, skv_local], BF16, tag="vTl")
                vtl = opool.tile([128, nkt_loc * 128], BF16, tag="vtl")
                kT = opool.tile([128, skv_tot], BF16, tag="kT")
                vfull = opool.tile([128, nkt * VSLOT], BF16, tag="vfull")

                def kslice(kt):
                    if not pair_split:
                        return kTl[:, kt * 128 : (kt + 1) * 128]
                    return kT[:, kt * 128 : (kt + 1) * 128]

                class Proj:
                    """Projection with DMA issue decoupled from compute.

                    Blocks in fp32_blocks load uncast (issuable on the
                    sync/scalar hwdge queues, avoiding the gpsimd-only cast
                    queue) and are transposed on the PE in fp32 (2
                    cycles/row); the transpose still writes bf16 to PSUM so
                    everything downstream is identical.
                    """

                    def __init__(self, x_dram, s_len, w_name, dstT, bias_ap,
                                 fp32_blocks=frozenset()):
                        self.x_dram = x_dram
                        self.nblk = s_len // 512
                        self.w_name = w_name
                        self.dstT = dstT
                        self.bias_ap = bias_ap
                        self.fp32_blocks = fp32_blocks
                        self.xs = [None] * self.nblk

                    def issue(self, sb, eng):
                        if sb in self.fp32_blocks:
                            xs = x32pool.tile([128, 4 * D_IN], FP32, tag="x32")
                        else:
                            xs = xpool.tile([128, 4 * D_IN], BF16, tag="xload")
                        r0 = sb * 512
                        eng.dma_start(
                            out=xs[:].rearrange("p (s i) -> p s i", s=4),
                            in_=self.x_dram[r0 : r0 + 512, :].rearrange(
                                "(s p) i -> p s i", s=4
                            ),
                        )
                        self.xs[sb] = xs

                    def gen(self):
                        for sb in range(self.nblk):
                            fp32 = sb in self.fp32_blocks
                            w_tiles = w_sb[self.w_name]
                            xs = self.xs[sb]
                            assert xs is not None, "xs DMA not issued"
                            xts = []
                            for icp in range(n_ic // 2):
                                xt_sb = xtpool.tile([128, 1024], BF16, tag="xt")
                                if fp32:
                                    # fp32 transposes must write fp32 PSUM:
                                    # half-width nuggets, one bank each.
                                    for half in range(2):
                                        ic = 2 * icp + half
                                        tp = pspool.tile(
                                            [128, 512], FP32, tag="ps"
                                        )
                                        for ss in range(4):
                                            nc.tensor.transpose(
                                                tp[:, ss * 128 : (ss + 1) * 128],
                                                xs[
                                                    :,
                                                    ss * D_IN
                                                    + ic * 128 : ss * D_IN
                                                    + (ic + 1) * 128,
                                                ],
                                                ident32[:],
                                            )
                                        nc.vector.tensor_copy(
                                            xt_sb[
                                                :, half * 512 : (half + 1) * 512
                                            ],
                                            tp[:],
                                        )
                                        yield
                                else:
                                    # two bf16 chunks share one PSUM bank
                                    tp = pspool.tile([128, 1024], BF16, tag="ps")
                                    for half in range(2):
                                        ic = 2 * icp + half
                                        for ss in range(4):
                                            nc.tensor.transpose(
                                                tp[
                                                    :,
                                                    half * 512
                                                    + ss * 128 : half * 512
                                                    + (ss + 1) * 128,
                                                ],
                                                xs[
                                                    :,
                                                    ss * D_IN
                                                    + ic * 128 : ss * D_IN
                                                    + (ic + 1) * 128,
                                                ],
                                                ident[:],
                                            )
                                    nc.vector.tensor_copy(xt_sb[:], tp[:])
                                    yield
                                xts.append(xt_sb[:, 0:512])
                                xts.append(xt_sb[:, 512:1024])
                            pp = pspool.tile([128, 512], FP32, tag="ps")
                            for ic in range(n_ic):
                                nc.tensor.matmul(
                                    pp[:],
                                    w_tiles[ic],
                                    xts[ic],
                                    start=(ic == 0),
                                    stop=(ic == n_ic - 1),
                                )
                            dst = self.dstT[:, sb * 512 : (sb + 1) * 512]
                            if self.bias_ap is None:
                                nc.vector.tensor_copy(dst, pp[:])
                            else:
                                nc.vector.tensor_scalar_add(
                                    dst, pp[:], self.bias_ap
                                )
                            yield

                # DRAM staging for the V exchange (one [128,512] DMA per
                # vtrans nugget, via the vtl SBUF staging tile). One
                # contiguous DRAM tile per exchange part: collectives
                # require contiguous input patterns.
                cc_in_v = [
                    dpool.tile(
                        [128, 4 * 128], BF16, tag=f"cc_in_v{p}",
                        name=f"cc_in_v{p}",
                    )
                    for p in range(max(nkt_loc // 4, 1))
                ]

                def vtrans_gen():
                    """Natural-layout local v tiles from vT (PE transposes),
                    shipped to DRAM exchange staging (pair modes) or copied
                    into vfull's local slots (standalone)."""
                    for kt4 in range(nkt_loc // 4):
                        tp = pspool.tile([128, 1024], BF16, tag="ps")
                        for j in range(4):
                            kt = kt4 * 4 + j
                            nc.tensor.transpose(
                                tp[:, j * 128 : (j + 1) * 128],
                                vTl[:, kt * 128 : (kt + 1) * 128],
                                ident[:],
                            )
                        if pair_split:
                            nc.vector.tensor_copy(
                                vtl[:, kt4 * 512 : (kt4 + 1) * 512], tp[:, 0:512]
                            )
                            nc.sync.dma_start(
                                out=cc_in_v[kt4][:, :],
                                in_=vtl[:, kt4 * 512 : (kt4 + 1) * 512],
                            )
                        else:
                            nc.vector.tensor_copy(
                                vfull[
                                    :, kt4 * 4 * VSLOT : (kt4 * 4 + 4) * VSLOT
                                ].rearrange("p (j s) -> p j s", j=4)[:, :, 0:128],
                                tp[:, 0:512].rearrange(
                                    "p (j d) -> p j d", j=4
                                ),
                            )
                        yield

                # P^T staging: one tile per query block, 16 ktp x 1024 cols.
                pt_tiles = {}

                def pt_tile(qb):
                    if qb not in pt_tiles:
                        t = ptpool.tile(
                            [128, (nkt // 2) * 1024], BF16, tag="pt",
                            name=f"pt{qb}",
                        )
                        pt_tiles[qb] = t
                    return pt_tiles[qb]

                # K-exchange part p carries kTl cols [p*512,(p+1)*512) and
                # lands as score pairs {2p,2p+1} (rank 0) and {8+2p,8+2p+1}
                # (rank 1). Order score steps by part arrival.
                if pair_split:
                    SC_ORDER = [
                        ktp
                        for p in range(2)
                        for ktp in (
                            4 * p, 4 * p + 1, 4 * p + 2, 4 * p + 3,
                            8 + 4 * p, 8 + 4 * p + 1, 8 + 4 * p + 2,
                            8 + 4 * p + 3,
                        )
                    ]
                else:
                    SC_ORDER = list(range(nkt // 2))

                # V-exchange part p carries local key tiles [8p, 8p+8) and
                # lands as global tiles {8p..} (rank 0) and {16+8p..}
                # (rank 1): iterate AV over keys in part-arrival order.
                if pair_split:
                    AV_ORDER = [
                        kt
                        for p in range(4)
                        for kt in list(range(4 * p, 4 * p + 4))
                        + list(range(16 + 4 * p, 16 + 4 * p + 4))
                    ]
                else:
                    AV_ORDER = list(range(nkt))

                def sc_gen(qb):
                    """Scores + exp for one query block; yields per ktp."""
                    pt = pt_tile(qb)
                    for ktp in SC_ORDER:
                        sc = scpool.tile([128, 1024], FP32, tag="sc")
                        for half in range(2):
                            kt = 2 * ktp + half
                            nc.tensor.matmul(
                                sc[:, half * 512 : (half + 1) * 512],
                                kslice(kt),
                                qT[:, qb * 512 : (qb + 1) * 512],
                                start=True,
                                stop=True,
                            )
                        nc.scalar.activation(
                            pt[:, ktp * 1024 : (ktp + 1) * 1024],
                            sc[:],
                            mybir.ActivationFunctionType.Exp,
                            bias=0.0,
                            scale=scale,
                        )
                        yield

                def av_gen(qb):
                    """P@V accumulation + normalize + store for one query
                    block; yields every 2 key tiles."""
                    pt = pt_tile(qb)
                    # accumulators packed 3+1 into two banks, exploiting that
                    # matmul start=True zeroes the WHOLE bank: qs0's start
                    # zeroes bank A (incl. qs1/qs2 regions, which then
                    # accumulate with start=False onto zeros); qs3's start
                    # zeroes bank B.
                    avA = pspool.tile([128, 3 * 129], FP32, tag="ps", name="avA")
                    avB = pspool.tile([128, 129], FP32, tag="ps", name="avB")
                    avs = [
                        avA[:, 0:129],
                        avA[:, 129:258],
                        avA[:, 258:387],
                        avB[:, 0:129],
                    ]
                    for i, kt in enumerate(AV_ORDER):
                        ktp, half = kt // 2, kt % 2
                        for qs in range(4):
                            nc.tensor.matmul(
                                avs[qs],
                                pt[
                                    :,
                                    ktp * 1024
                                    + half * 512
                                    + qs * 128 : ktp * 1024
                                    + half * 512
                                    + (qs + 1) * 128,
                                ],
                                vfull[:, kt * VSLOT : kt * VSLOT + 129],
                                start=(i == 0 and qs in (0, 3)),
                                stop=(i == nkt - 1),
                                skip_group_check=(i == 0 and qs in (1, 2)),
                            )
                        if i % 2 == 1:
                            yield
                    obuf = finpool.tile([128, 4 * D], FP32, tag="obuf")
                    for qs in range(4):
                        rec = finpool.tile([128, 1], FP32, tag="rec")
                        nc.vector.reciprocal(rec[:], avs[qs][:, 128:129])
                        nc.vector.tensor_scalar_mul(
                            obuf[:, qs * D : (qs + 1) * D], avs[qs][:, 0:128], rec[:]
                        )
                    r0 = qb * 512
                    nc.sync.dma_start(
                        out=out[r0 : r0 + 512, :].rearrange("(s p) d -> p s d", s=4),
                        in_=obuf[:].rearrange("p (s d) -> p s d", s=4),
                    )
                    yield

                groups = [[2 * i, 2 * i + 1] for i in range(n_cores // 2)]

                def exchange(src_ap, src_is_dram, unpack_aps, tag, eng_a, eng_b):
                    """Ship a [128, w] slice across the core pair;
                    unpack_aps[h] receives rank h's copy. Mock mode bounces
                    the data through DRAM (equivalent transfer volume) with
                    hops split across two queues so parts pipeline."""
                    w = src_ap.shape[-1]
                    if src_is_dram:
                        cc_in = src_ap
                    else:
                        t = dpool.tile(
                            [128, w], BF16, tag=f"cc_in_{tag}",
                            name=f"cc_in_{tag}",
                        )
                        eng_a.dma_start(out=t[:], in_=src_ap)
                        cc_in = t[:]
                    if pair_split == "mock":
                        cc_b = dpool.tile(
                            [128, w], BF16, tag=f"cc_b_{tag}", name=f"cc_b_{tag}"
                        )
                        hw = w // 2
                        eng_a.dma_start(out=cc_b[:, 0:hw], in_=cc_in[:, 0:hw])
                        eng_b.dma_start(out=cc_b[:, hw:w], in_=cc_in[:, hw:w])
                        halves = [cc_b[:], cc_b[:]]
                    else:
                        cc_out = dpool.tile(
                            [2, 128, w], BF16, tag=f"cc_o_{tag}",
                            name=f"cc_o_{tag}",
                        )
                        nc.gpsimd.collective_compute(
                            "AllGather",
                            mybir.AluOpType.bypass,
                            replica_groups=groups,
                            ins=[cc_in.opt()],
                            outs=[cc_out.opt()],
                        )
                        halves = [cc_out[0], cc_out[1]]
                    for h, eng in ((0, eng_a), (1, eng_b)):
                        eng.dma_start(out=unpack_aps[h], in_=halves[h])

                def k_exchange(part):
                    """Exchange part p = 1024 projected-K^T columns. The
                    scalar (ACT) queue only carries hops that finish before
                    the exp stream starts."""
                    c0 = part * 1024
                    eng_a, eng_b = [
                        (nc.sync, nc.scalar),
                        (nc.scalar, nc.sync),
                    ][part]
                    exchange(
                        kTl[:, c0 : c0 + 1024],
                        False,
                        [
                            kT[:, h * skv_local + c0 : h * skv_local + c0 + 1024]
                            for h in range(2)
                        ],
                        f"k{part}",
                        eng_a,
                        eng_b,
                    )

                def v_exchange(part):
                    """Exchange part p = 4 local natural-layout v tiles
                    (one vtrans nugget). Hops on sync: scalar carries exp,
                    gpsimd carries the input casts."""
                    exchange(
                        cc_in_v[part][:, :],
                        True,
                        [
                            vfull[
                                :,
                                (h * nkt_loc + part * 4)
                                * VSLOT : (h * nkt_loc + part * 4 + 4)
                                * VSLOT,
                            ].rearrange("p (j s) -> p j s", j=4)[:, :, 0:128]
                            for h in range(2)
                        ],
                        f"v{part}",
                        nc.sync,
                        nc.sync,
                    )

                def drain(g):
                    for _ in g:
                        pass

                def advance(g, n):
                    took = 0
                    try:
                        for _ in range(n):
                            next(g)
                            took += 1
                    except StopIteration:
                        pass
                    return took

                # ---- emission schedule ----
                bq_sb = cpool.tile([128, 1], FP32, tag="bq")
                bv_sb = cpool.tile([128, 1], FP32, tag="bv")
                kp = Proj(k_in, skv_local, "wk", kTl, None)
                qp = Proj(q_in, sq, "wq", qT, bq_sb[:])
                vp = Proj(v_in, skv_local, "wv", vTl, bv_sb[:])
                kgen, qgen, vgen = kp.gen(), qp.gen(), vp.gen()
                vtrans = vtrans_gen()
                scs = [sc_gen(qb) for qb in range(nqb)]
                avs_g = [av_gen(qb) for qb in range(nqb)]

                # t0 DMA queueing, all casts on the single gpsimd queue.
                # The device is aggregate-HBM-bound (~275 GB/s effective
                # across 8 cores), so one queue is as good as three; order
                # is what matters. Blocks land every ~7us: K first (its
                # exchange gates all scores), Q0 next, then Q1/Q2 (each
                # gates its score phase), V spread so each V-exchange part
                # launches as its data lands, Q3 before V3.
                load_w("wk", wk)
                kp.issue(0, nc.gpsimd)
                kp.issue(1, nc.gpsimd)
                load_w("wq", wq)
                qp.issue(0, nc.gpsimd)
                kp.issue(2, nc.gpsimd)
                kp.issue(3, nc.gpsimd)
                qp.issue(1, nc.gpsimd)
                qp.issue(2, nc.gpsimd)
                vp.issue(0, nc.gpsimd)
                load_w("wv", wv)
                nc.sync.dma_start(out=bq_sb[:], in_=bq[:, :])
                nc.sync.dma_start(out=bv_sb[:], in_=bv[:, :])

                # warmups during the initial DMA window: PE p-state ramp via
                # identity transposes; ACT exp-table preload (1283ns) off the
                # critical exp stream.
                if WARMUP_TRANSPOSES:
                    wps = pspool.tile(
                        [128, 512], BF16, tag="ps", name="warm"
                    )
                    for i in range(WARMUP_TRANSPOSES):
                        nc.tensor.transpose(
                            wps[:, (i % 4) * 128 : (i % 4 + 1) * 128],
                            ident[:],
                            ident[:],
                        )
                actwarm = cpool.tile([128, 1], FP32, tag="actwarm")
                nc.vector.memset(actwarm[:], 0.0)
                actwarm2 = cpool.tile([128, 1], FP32, tag="actwarm2")
                nc.scalar.activation(
                    actwarm2[:],
                    actwarm[:],
                    mybir.ActivationFunctionType.Exp,
                    bias=0.0,
                    scale=1.0,
                )
                # ones columns of v_aug, one strided memset; disjoint from
                # the unpack DMAs' [0:128) slot regions.
                nc.vector.memset(
                    vfull[:, :].rearrange("p (j s) -> p j s", j=nkt)[
                        :, :, 128:129
                    ],
                    1.0,
                )

                # K blocks; exchange part A (cols 0:1024) after K1, part B
                # after K3. Q0 projects between them so qb0 scores start as
                # soon as part A lands.
                advance(kgen, 5)  # K block 0
                advance(kgen, 5)  # K block 1
                if pair_split:
                    k_exchange(0)
                advance(qgen, 5)  # Q block 0
                advance(kgen, 5)  # K block 2
                drain(kgen)  # K block 3
                if pair_split:
                    k_exchange(1)

                # qb0 scores: input-bound window, no fills (later blocks
                # haven't landed yet); the part-B tail of qb0 overlaps the
                # Q1 arrival gap.
                drain(scs[0])
                # Q1 projection gates qb1's scores.
                advance(qgen, 5)  # Q1
                vp.issue(1, nc.gpsimd)
                # qb1 scores: weave V0/V1 as they land; first two V-exchange
                # parts go out per vtrans nugget; Q2 last (lands ~54us).
                for i in range(5):
                    advance(scs[1], 1)
                    advance(vgen, 1)  # V0
                vp.issue(2, nc.gpsimd)
                for i in range(5):
                    advance(scs[1], 1)
                    advance(vgen, 1)  # V1
                qp.issue(3, nc.gpsimd)
                for i in range(2):
                    advance(scs[1], 1)
                    advance(vtrans, 1)  # vtrans n0, n1
                    if pair_split:
                        v_exchange(i)
                for i in range(4):
                    advance(scs[1], 1)
                    advance(qgen, 1)  # Q2 n0-3
                advance(qgen, 1)  # Q2 n4 (gates qb2 scores)
                vp.issue(3, nc.gpsimd)
                # qb2 scores: V2/V3 land here; exchange parts 2-3 per
                # nugget; AV(qb0) part-0 chunks weave in once vfull part 0
                # lands (~80us).
                for i in range(5):
                    advance(scs[2], 1)
                    advance(vgen, 1)  # V2
                for i in range(5):
                    advance(scs[2], 1)
                    advance(vgen, 1)  # V3
                for i in range(2):
                    advance(scs[2], 1)
                    advance(vtrans, 1)  # vtrans n2, n3
                    if pair_split:
                        v_exchange(2 + i)
                drain(vtrans)
                drain(vgen)
                for i in range(4):
                    advance(scs[2], 1)
                    advance(avs_g[0], 1)
                # Q3 projection gates qb3's scores.
                drain(qgen)  # Q3
                # qb3 scores: drain AV(qb0) at double rate (frees the pt
                # buffer qb3's exps rotate onto), then AV(qb1).
                for i in range(16):
                    advance(scs[3], 1)
                    if i < 7:
                        advance(avs_g[0], 2)
                    else:
                        advance(avs_g[1], 1)
                for g in avs_g:
                    drain(g)

            if reps > 1:
                hint = (
                    mybir.EngineType.PE,
                    mybir.EngineType.DVE,
                    mybir.EngineType.Activation,
                    mybir.EngineType.SP,
                    mybir.EngineType.Pool,
                )
                with tc.For_i(0, reps, 1, hint_engines=hint):
                    emit_body()
            else:
                emit_body()

    return nc


def build_graph(
    sq=S // 2, skv_local=S // 2, n_cores=N_CORES, pair_split=True, reps=1
):
    nc = bacc.Bacc(
        "TRN2",
        target_bir_lowering=False,
        debug=False,
        enable_asserts=True,
        num_devices=n_cores,
    )
    build_program(
        nc, sq, skv_local, n_cores=n_cores, pair_split=pair_split, reps=reps
    )
    nc.compile()
    return nc


_NC = None


def _get_nc():
    global _NC
    if _NC is None:
        _NC = build_graph()
    return _NC


def make_in_maps(query, key, value, Wq, bq, Wk, bk, Wv, bv):
    query = np.asarray(query, dtype=np.float32)
    key = np.asarray(key, dtype=np.float32)
    value = np.asarray(value, dtype=np.float32)
    Wq = np.ascontiguousarray(np.asarray(Wq, dtype=np.float32))
    Wk = np.ascontiguousarray(np.asarray(Wk, dtype=np.float32))
    Wv = np.ascontiguousarray(np.asarray(Wv, dtype=np.float32))
    bq2 = np.ascontiguousarray(np.asarray(bq, np.float32).reshape(D, 1))
    bv2 = np.ascontiguousarray(np.asarray(bv, np.float32).reshape(D, 1))
    sq = S // 2
    in_maps = []
    for c in range(N_CORES):
        b, h = c // 2, c % 2
        in_maps.append(
            {
                "q_in": np.ascontiguousarray(query[b, h * sq : (h + 1) * sq, :]),
                "k_in": np.ascontiguousarray(key[b, h * sq : (h + 1) * sq, :]),
                "v_in": np.ascontiguousarray(value[b, h * sq : (h + 1) * sq, :]),
                "wq": Wq,
                "wk": Wk,
                "wv": Wv,
                "bq": bq2,
                "bv": bv2,
            }
        )
    return in_maps


def assemble_out(results):
    sq = S // 2
    out = np.empty((B, S, D), np.float32)
    for c in range(N_CORES):
        b, h = c // 2, c % 2
        out[b, h * sq : (h + 1) * sq, :] = results[c]["out"]
    return out


def kernel(query, key, value, Wq, bq, Wk, bk, Wv, bv):
    nc = _get_nc()
    in_maps = make_in_maps(query, key, value, Wq, bq, Wk, bk, Wv, bv)
    res = run_bass_kernel_spmd(nc, in_maps, core_ids=list(range(N_CORES)))
    return assemble_out(res.results)
